# revision 35
# baseline (speedup 1.0000x reference)
"""Multi-head attention forward for TRN2, 8 NeuronCores, data-parallel over batch.

Reference computation (B=16, S=1024, D=768, H=12, HD=64), fp32:
    q = einsum('bsd,dhe->bshe', x, Wq) + bq        (same for k, v)
    z = einsum('bqhd,bkhd->bhqk', q/8, k)
    a = softmax(z, axis=-1)
    o = einsum('bhqk,bkhd->bqhd', a, v)
    y = einsum('bqhd,hde->bqe', o, Wo) + bo

Design (per core, 2 batches, phases pipelined by the Tile scheduler):
  - One orientation flip at input: x [S,D] -> xT [D,S] via PE transpose
    (fp32 is_transpose matmuls, 6 per seq tile into one 2-bank PSUM tile).
  - Projections produce QT,KT [D,S] (head-transposed) and V [S,D] directly
    from xT; all feed-forward tensors are written as float32r by the
    evicting engine (BIR requires fp32r matmul inputs to be rounded by
    their producer).
  - Scores computed transposed: zT[k,q] = KT_slice.T @ QT_slice
    (contraction=64). Heads are processed in pairs: the even/odd head's
    score matmuls sit at PE row groups 0-63/64-127 (tile_position derived
    from base_partition) as adjacent instructions, so the hardware runs
    them concurrently (row-tiling).
  - exp on ACT with scale=1/8 fused; no max-subtraction needed (|z|<~3).
  - PV: U_ext[0:65,q] = sum_k Vext[k,0:65].T @ expZT[k,q]; Vext carries a
    ones column so the softmax denominator accumulates in PSUM row 64.
  - U+denom evicted to SBUF immediately (frees the PSUM accumulator), then:
    DMA partition-broadcast of the denominator row (gpsimd SWDGE queue),
    reciprocal_approx_fast on DVE, DVE tensor_mul -> OTn [D,S], which is
    exactly the out-projection stationary layout. Odd heads are
    DMA-shifted to partitions 64-127 (DVE lanes cannot cross partitions).
  - y[q,d] = sum_c OTn[c,q-128].T @ Wo[c,d] (+ ones x cvec rank-1 when
    biases are nonzero; cvec = bv@Wo + bo; bq/bk fold into the QT/KT
    evictions; bv/bo commute through softmax normalization exactly).
  - All matmuls in float32r (tf32-like, ~1.4e-4 rel err, 1 cycle/row at
    N>=256 vs 4 for fp32). End-to-end rel err vs fp32 reference: 2.6e-4.
  - Big DMAs batched as [128, 2, 768] tile pairs on the sync queue;
    phase-C DMAs ride the gpsimd queue to keep the sync queue clear.
  - TimelineSim cost model: ~412 us per core.
"""

import numpy as np
from contextlib import ExitStack

import concourse.bacc as bacc
import concourse.bass as bass
import concourse.tile as tile
import concourse.mybir as mybir
from concourse.bass_utils import run_bass_kernel_spmd
from concourse.masks import make_identity

B, S, D, H, HD = 16, 1024, 768, 12, 64
NCORES = 8
BL = B // NCORES      # batches per core
P = 128
DC = D // P           # 6 contraction chunks
SQ = S // P           # 8 seq tiles of 128
F32 = mybir.dt.float32
F32R = mybir.dt.float32r
EXP = mybir.ActivationFunctionType.Exp
SCALE = 1.0 / float(np.sqrt(HD))

_NC = {}
_DEBUG = False  # add DRAM dumps of intermediates (batch 0)


def _emit(tc, x_d, w_d, b_d, y_d, dbg=None, with_bias=True):
    """Emit the whole per-core program. w_d/b_d: dicts of DRAM APs."""
    nc = tc.nc

    def dump(name, sbuf_ap):
        if dbg is not None and name in dbg:
            nc.sync.dma_start(out=dbg[name], in_=sbuf_ap)

    with ExitStack() as ctx:
        consts = ctx.enter_context(tc.tile_pool(name="consts", bufs=1))
        wpool = ctx.enter_context(tc.tile_pool(name="wpool", bufs=2))
        big = ctx.enter_context(tc.tile_pool(name="big", bufs=1))
        atp = ctx.enter_context(
            tc.tile_pool(name="atp", bufs=(2 if with_bias else 4)))
        iop = ctx.enter_context(tc.tile_pool(name="iop", bufs=3))
        smal = ctx.enter_context(tc.tile_pool(name="smal", bufs=2))
        pp = ctx.enter_context(tc.tile_pool(name="pp", bufs=2, space="PSUM"))

        # ---- constants ----
        ident = consts.tile([P, P], F32)
        make_identity(nc, ident)
        if with_bias:
            bq_sb = consts.tile([P, DC], F32)
            nc.sync.dma_start(out=bq_sb,
                              in_=b_d["bq"].rearrange("(c p) -> p c", p=P))
            bk_sb = consts.tile([P, DC], F32)
            nc.sync.dma_start(out=bk_sb,
                              in_=b_d["bk"].rearrange("(c p) -> p c", p=P))
            bv_st = consts.tile([P, DC], F32)
            nc.sync.dma_start(out=bv_st,
                              in_=b_d["bv"].rearrange("(c p) -> p c", p=P))
            bv_r = consts.tile([P, DC], F32R)
            nc.vector.tensor_copy(bv_r, bv_st)
            bo_st = consts.tile([1, D], F32)
            nc.sync.dma_start(out=bo_st, in_=b_d["bo"].unsqueeze(0))
            bo_r = consts.tile([1, D], F32R)
            nc.vector.tensor_copy(bo_r, bo_st)
            ones_f32 = consts.tile([1, P], F32)
            nc.vector.memset(ones_f32, 1.0)
            ones_row_r = consts.tile([1, P], F32R)
            nc.vector.tensor_copy(ones_row_r, ones_f32)
            cvec_sb = consts.tile([1, D], F32R)
        ones96 = consts.tile([P, SQ * H], F32)
        nc.vector.memset(ones96, 1.0)
        cvec_done = False

        def load_weight(name):
            # DMA fp32 2-chunk pairs through staging, round to f32r on gpsimd
            wr = wpool.tile([P, DC, D], F32R, tag="w", name=f"w_{name}")
            src = w_d[name].rearrange("(c p) m -> p c m", p=P)
            for c in range(0, DC, 2):
                ws = iop.tile([P, 2, D], F32, tag="st2", name=f"ws_{name}_{c}")
                nc.sync.dma_start(out=ws, in_=src[:, c:c + 2, :])
                nc.vector.tensor_copy(wr[:, c:c + 2, :], ws)
            return wr

        for b in range(BL):
            x_b = x_d[b].rearrange("(t p) d -> p t d", p=P)
            y_b = y_d[b].rearrange("(t p) d -> p t d", p=P)

            # ---- phase A: x -> xT (f32r) ----
            xT = big.tile([P, DC, S], F32R, tag="xT", name=f"xT_{b}")
            for sq in range(0, SQ, 2):
                x_in = iop.tile([P, 2, D], F32, tag="st2", name=f"xin_{b}_{sq}")
                nc.sync.dma_start(out=x_in, in_=x_b[:, sq:sq + 2, :])
                for j in range(2):
                    tt = pp.tile([P, 1024], F32, tag="mm",
                                 name=f"tps_{b}_{sq}_{j}")
                    for c in range(DC):
                        nc.tensor.transpose(
                            tt[:, c * P:(c + 1) * P],
                            x_in[:, j, c * P:(c + 1) * P], ident
                        )
                    nc.vector.tensor_copy(
                        xT[:, :, (sq + j) * P:(sq + j + 1) * P],
                        tt[:, :D].rearrange("p (c q) -> p c q", c=DC),
                    )

            if b == 0:
                dump("xT", xT)

            # ---- phase B: projections ----
            wq_r = load_weight("wq")
            QT = big.tile([P, DC, S], F32R, tag="QT", name=f"QT_{b}")
            for m in range(DC):
                qq = pp.tile([P, 1024], F32, tag="mm", name=f"qps_{b}_{m}")
                for c in range(DC):
                    for hf in range(2):
                        nc.tensor.matmul(
                            qq[:, hf * 512:(hf + 1) * 512],
                            wq_r[:, c, m * P:(m + 1) * P],
                            xT[:, c, hf * 512:(hf + 1) * 512],
                            start=(c == 0), stop=(c == DC - 1),
                        )
                if with_bias:
                    nc.vector.tensor_scalar_add(QT[:, m, :], qq,
                                                bq_sb[:, m:m + 1])
                else:
                    nc.vector.tensor_copy(QT[:, m, :], qq)

            wk_r = load_weight("wk")
            KT = big.tile([P, DC, S], F32R, tag="KT", name=f"KT_{b}")
            for m in range(DC):
                kk = pp.tile([P, 1024], F32, tag="mm", name=f"kps_{b}_{m}")
                for c in range(DC):
                    for hf in range(2):
                        nc.tensor.matmul(
                            kk[:, hf * 512:(hf + 1) * 512],
                            wk_r[:, c, m * P:(m + 1) * P],
                            xT[:, c, hf * 512:(hf + 1) * 512],
                            start=(c == 0), stop=(c == DC - 1),
                        )
                if with_bias:
                    nc.vector.tensor_scalar_add(KT[:, m, :], kk,
                                                bk_sb[:, m:m + 1])
                else:
                    nc.vector.tensor_copy(KT[:, m, :], kk)

            wv_r = load_weight("wv")
            # V layout [P, SQ, H, 65]: cols 0..63 = v, col 64 = ones
            V = big.tile([P, SQ, H, 65], F32R, tag="V", name=f"V_{b}")
            nc.vector.tensor_copy(
                V[:, :, :, 64], ones96.rearrange("p (a h) -> p a h", a=SQ)
            )
            for sq in range(SQ):
                vv = pp.tile([P, 1024], F32, tag="mm", name=f"vps_{b}_{sq}")
                for c in range(DC):
                    nc.tensor.matmul(
                        vv[:, 0:512], xT[:, c, sq * P:(sq + 1) * P],
                        wv_r[:, c, 0:512], start=(c == 0), stop=(c == DC - 1),
                    )
                    nc.tensor.matmul(
                        vv[:, 512:D], xT[:, c, sq * P:(sq + 1) * P],
                        wv_r[:, c, 512:D], start=(c == 0), stop=(c == DC - 1),
                    )
                nc.vector.tensor_copy(
                    V[:, sq, :, 0:64],
                    vv[:, :D].rearrange("p (h e) -> p h e", h=H),
                )
            if b == 0:
                dump("QT", QT)
                dump("KT", KT)
                dump("V", V)

            # ---- phase C: attention, head pairs interleaved ----
            # scores of the even/odd head hit PE row groups 0-63/64-127 as
            # adjacent instructions so the hardware overlaps them; U+denom
            # are evicted to SBUF immediately to free the PSUM accumulators.
            OTn = big.tile([P, DC, S], F32R, tag="OTn", name=f"OTn_{b}")
            for ch in range(DC):
                oos = [pp.tile([P, 1024], F32, tag="ov",
                               name=f"ops_{b}_{ch}_{par}")
                       for par in range(2)]
                for kt in range(SQ):
                    zzs = []
                    for par in range(2):
                        psl = slice(par * 64, par * 64 + 64)
                        zz = pp.tile([P, 1024], F32, tag="mm",
                                     name=f"zps_{b}_{ch}_{par}_{kt}")
                        ksl = KT[psl, ch, kt * P:(kt + 1) * P]
                        nc.tensor.matmul(zz[:, 0:512], ksl,
                                         QT[psl, ch, 0:512],
                                         start=True, stop=True)
                        nc.tensor.matmul(zz[:, 512:1024], ksl,
                                         QT[psl, ch, 512:1024],
                                         start=True, stop=True)
                        zzs.append(zz)
                    ats = []
                    for par in range(2):
                        at = atp.tile([P, 1024], F32R, tag="at",
                                      name=f"at_{b}_{ch}_{par}_{kt}")
                        nc.scalar.activation(at, zzs[par], EXP, scale=SCALE)
                        ats.append(at)
                        if b == 0 and ch == 0 and par == 0 and kt == 0:
                            dump("at0", at)
                    for par in range(2):
                        vsl = V[:, kt, 2 * ch + par, :]
                        for hf in range(2):
                            nc.tensor.matmul(
                                oos[par][0:65, hf * 512:(hf + 1) * 512],
                                vsl, ats[par][:, hf * 512:(hf + 1) * 512],
                                start=(kt == 0), stop=(kt == SQ - 1),
                            )
                # evict U+denom, then normalize from SBUF
                ues = []
                for par in range(2):
                    ue = smal.tile([65, S], F32, tag="ub",
                                   name=f"ue_{b}_{ch}_{par}", bufs=2)
                    nc.vector.tensor_copy(ue, oos[par][0:65, :])
                    ues.append(ue)
                for par in range(2):
                    h = 2 * ch + par
                    psl = slice(par * 64, par * 64 + 64)
                    ue = ues[par]
                    dsl = ue[64:65, :]
                    rbraw = smal.tile([64, S], F32, tag="rbraw",
                                      name=f"rbraw_{b}_{h}", bufs=1)
                    srcap = bass.AP(tensor=dsl.tensor, offset=dsl.offset,
                                    ap=[list(dsl.ap[0]), [0, 64],
                                        list(dsl.ap[1])])
                    nc.gpsimd.dma_start(out=rbraw, in_=srcap)
                    rb = smal.tile([64, S], F32, tag="rb",
                                   name=f"rb_{b}_{h}", bufs=1)
                    nc.vector.reciprocal_approx_fast(out=rb, in_=rbraw)
                    if par == 0:
                        nc.vector.tensor_mul(OTn[psl, ch, :], ue[0:64, :], rb)
                    else:
                        stg = smal.tile([64, S], F32R, tag="rbraw",
                                        name=f"stg_{b}_{h}", bufs=1)
                        nc.vector.tensor_mul(stg, ue[0:64, :], rb)
                        nc.gpsimd.dma_start(out=OTn[psl, ch, :], in_=stg)
                    if b == 0 and h == 0:
                        dump("rbi0", rb)
            if b == 0:
                dump("OTn", OTn)

            # ---- phase D/E: output projection ----
            wo_r = load_weight("wo")
            if with_bias and not cvec_done:
                cvec_done = True
                cv = pp.tile([P, 1024], F32, tag="ov", name="cvps")
                for c in range(DC):
                    nc.tensor.matmul(cv[0:1, 0:512], bv_r[:, c:c + 1],
                                     wo_r[:, c, 0:512], start=(c == 0),
                                     stop=False)
                    nc.tensor.matmul(cv[0:1, 512:D], bv_r[:, c:c + 1],
                                     wo_r[:, c, 512:D], start=(c == 0),
                                     stop=False)
                nc.tensor.matmul(cv[0:1, 0:512], ones_row_r[:, 0:1],
                                 bo_r[:, 0:512], start=False, stop=True)
                nc.tensor.matmul(cv[0:1, 512:D], ones_row_r[:, 0:1],
                                 bo_r[:, 512:D], start=False, stop=True)
                nc.vector.tensor_copy(cvec_sb, cv[0:1, :D])

            for sq in range(0, SQ, 2):
                yst = iop.tile([P, 2, D], F32, tag="st2", name=f"yst_{b}_{sq}")
                for j in range(2):
                    yy = pp.tile([P, 1024], F32, tag="mm",
                                 name=f"yps_{b}_{sq}_{j}")
                    for c in range(DC):
                        st = OTn[:, c, (sq + j) * P:(sq + j + 1) * P]
                        last = (not with_bias) and c == DC - 1
                        nc.tensor.matmul(yy[:, 0:512], st, wo_r[:, c, 0:512],
                                         start=(c == 0), stop=last)
                        nc.tensor.matmul(yy[:, 512:D], st, wo_r[:, c, 512:D],
                                         start=(c == 0), stop=last)
                    if with_bias:
                        nc.tensor.matmul(yy[:, 0:512], ones_row_r,
                                         cvec_sb[:, 0:512], start=False,
                                         stop=True)
                        nc.tensor.matmul(yy[:, 512:D], ones_row_r,
                                         cvec_sb[:, 512:D], start=False,
                                         stop=True)
                    nc.vector.tensor_copy(yst[:, j, :], yy[:, :D])
                nc.sync.dma_start(out=y_b[:, sq:sq + 2, :], in_=yst)


def _build(with_bias=True):
    nc = bacc.Bacc("TRN2", target_bir_lowering=False, debug=False,
                   num_devices=NCORES)
    x_d = nc.dram_tensor("x", [BL, S, D], F32, kind="ExternalInput").ap()
    w_d = {n: nc.dram_tensor(n, [D, D], F32, kind="ExternalInput").ap()
           for n in ("wq", "wk", "wv", "wo")}
    b_d = {n: nc.dram_tensor(n, [D], F32, kind="ExternalInput").ap()
           for n in ("bq", "bk", "bv", "bo")}
    y_d = nc.dram_tensor("y", [BL, S, D], F32, kind="ExternalOutput").ap()
    dbg = None
    if _DEBUG:
        shapes = {"xT": ([P, DC, S], F32R), "QT": ([P, DC, S], F32R),
                  "KT": ([P, DC, S], F32R), "V": ([P, SQ, H, 65], F32R),
                  "at0": ([P, S], F32R), "rb0": ([65, S], F32),
                  "rbi0": ([64, S], F32),
                  "OTn": ([P, DC, S], F32R)}
        dbg = {n: nc.dram_tensor(f"dbg_{n}", sh, dt,
                                 kind="ExternalOutput").ap()
               for n, (sh, dt) in shapes.items()}
    with tile.TileContext(nc) as tc:
        _emit(tc, x_d, w_d, b_d, y_d, dbg, with_bias=with_bias)
    nc.compile()
    return nc


def _in_maps(x, Wq, bq, Wk, bk, Wv, bv, Wo, bo):
    w = {
        "wq": np.ascontiguousarray(Wq.reshape(D, D), dtype=np.float32),
        "wk": np.ascontiguousarray(Wk.reshape(D, D), dtype=np.float32),
        "wv": np.ascontiguousarray(Wv.reshape(D, D), dtype=np.float32),
        "wo": np.ascontiguousarray(Wo.reshape(D, D), dtype=np.float32),
        "bq": np.ascontiguousarray(bq.reshape(D), dtype=np.float32),
        "bk": np.ascontiguousarray(bk.reshape(D), dtype=np.float32),
        "bv": np.ascontiguousarray(bv.reshape(D), dtype=np.float32),
        "bo": np.ascontiguousarray(bo.reshape(D), dtype=np.float32),
    }
    x = np.asarray(x, dtype=np.float32)
    return [dict(w, x=np.ascontiguousarray(x[i * BL:(i + 1) * BL]))
            for i in range(NCORES)]


def get_nc(with_bias=True):
    if with_bias not in _NC:
        _NC[with_bias] = _build(with_bias=with_bias)
    return _NC[with_bias]


def run(inputs, trace=False):
    with_bias = any(
        np.any(np.asarray(inputs[k])) for k in ("bq", "bk", "bv", "bo"))
    nc = get_nc(with_bias=with_bias)
    maps = _in_maps(**inputs)
    res = run_bass_kernel_spmd(nc, maps, list(range(NCORES)), trace=trace)
    y = np.concatenate([res.results[i]["y"] for i in range(NCORES)], axis=0)
    return y, res


def kernel(x, Wq, bq, Wk, bk, Wv, bv, Wo, bo):
    y, _ = run(dict(x=x, Wq=Wq, bq=bq, Wk=Wk, bk=bk, Wv=Wv, bv=bv,
                    Wo=Wo, bo=bo))
    return y


# revision 43
# speedup vs baseline: 1.0005x; 1.0005x over previous
"""Multi-head attention forward for TRN2, 8 NeuronCores, data-parallel over batch.

Reference computation (B=16, S=1024, D=768, H=12, HD=64), fp32:
    q = einsum('bsd,dhe->bshe', x, Wq) + bq        (same for k, v)
    z = einsum('bqhd,bkhd->bhqk', q/8, k)
    a = softmax(z, axis=-1)
    o = einsum('bhqk,bkhd->bqhd', a, v)
    y = einsum('bqhd,hde->bqe', o, Wo) + bo

Design (per core, 2 batches, phases pipelined by the Tile scheduler):
  - One orientation flip at input: x [S,D] -> xT [D,S] via PE transpose
    (fp32 is_transpose matmuls, 6 per seq tile into one 2-bank PSUM tile).
  - Projections produce QT,KT [D,S] (head-transposed) and V [S,D] directly
    from xT; all feed-forward tensors are written as float32r by the
    evicting engine (BIR requires fp32r matmul inputs to be rounded by
    their producer).
  - Scores computed transposed: zT[k,q] = KT_slice.T @ QT_slice
    (contraction=64). Heads are processed in pairs: the even/odd head's
    score matmuls sit at PE row groups 0-63/64-127 (tile_position derived
    from base_partition) as adjacent instructions, so the hardware runs
    them concurrently (row-tiling).
  - exp on ACT with scale=1/8 fused; no max-subtraction needed (|z|<~3).
  - PV: U_ext[0:65,q] = sum_k Vext[k,0:65].T @ expZT[k,q]; Vext carries a
    ones column so the softmax denominator accumulates in PSUM row 64.
  - U+denom evicted to SBUF immediately (frees the PSUM accumulator), then:
    DMA partition-broadcast of the denominator row (gpsimd SWDGE queue),
    reciprocal_approx_fast on DVE, DVE tensor_mul -> OTn [D,S], which is
    exactly the out-projection stationary layout. Odd heads are
    DMA-shifted to partitions 64-127 (DVE lanes cannot cross partitions).
  - y[q,d] = sum_c OTn[c,q-128].T @ Wo[c,d] (+ ones x cvec rank-1 when
    biases are nonzero; cvec = bv@Wo + bo; bq/bk fold into the QT/KT
    evictions; bv/bo commute through softmax normalization exactly).
  - All matmuls in float32r (tf32-like, ~1.4e-4 rel err, 1 cycle/row at
    N>=256 vs 4 for fp32). End-to-end rel err vs fp32 reference: 2.6e-4.
  - Big DMAs batched as [128, 2, 768] tile pairs on the sync queue;
    phase-C DMAs ride the gpsimd queue to keep the sync queue clear.
  - TimelineSim cost model: ~412 us per core.
"""

import numpy as np
from contextlib import ExitStack

import concourse.bacc as bacc
import concourse.bass as bass
import concourse.tile as tile
import concourse.mybir as mybir
from concourse.bass_utils import run_bass_kernel_spmd
from concourse.masks import make_identity

B, S, D, H, HD = 16, 1024, 768, 12, 64
NCORES = 8
BL = B // NCORES      # batches per core
P = 128
DC = D // P           # 6 contraction chunks
SQ = S // P           # 8 seq tiles of 128
F32 = mybir.dt.float32
F32R = mybir.dt.float32r
EXP = mybir.ActivationFunctionType.Exp
SCALE = 1.0 / float(np.sqrt(HD))

_NC = {}
_DEBUG = False  # add DRAM dumps of intermediates (batch 0)


def _emit(tc, x_d, w_d, b_d, y_d, dbg=None, with_bias=True):
    """Emit the whole per-core program. w_d/b_d: dicts of DRAM APs."""
    nc = tc.nc

    def dump(name, sbuf_ap):
        if dbg is not None and name in dbg:
            nc.sync.dma_start(out=dbg[name], in_=sbuf_ap)

    with ExitStack() as ctx:
        consts = ctx.enter_context(tc.tile_pool(name="consts", bufs=1))
        wpool = ctx.enter_context(tc.tile_pool(name="wpool", bufs=2))
        big = ctx.enter_context(tc.tile_pool(name="big", bufs=1))
        atp = ctx.enter_context(
            tc.tile_pool(name="atp", bufs=(2 if with_bias else 3)))
        iop = ctx.enter_context(tc.tile_pool(name="iop", bufs=3))
        smal = ctx.enter_context(tc.tile_pool(name="smal", bufs=2))
        pp = ctx.enter_context(tc.tile_pool(name="pp", bufs=2, space="PSUM"))

        # ---- constants ----
        ident = consts.tile([P, P], F32)
        make_identity(nc, ident)
        if with_bias:
            bq_sb = consts.tile([P, DC], F32)
            nc.sync.dma_start(out=bq_sb,
                              in_=b_d["bq"].rearrange("(c p) -> p c", p=P))
            bk_sb = consts.tile([P, DC], F32)
            nc.sync.dma_start(out=bk_sb,
                              in_=b_d["bk"].rearrange("(c p) -> p c", p=P))
            bv_st = consts.tile([P, DC], F32)
            nc.sync.dma_start(out=bv_st,
                              in_=b_d["bv"].rearrange("(c p) -> p c", p=P))
            bv_r = consts.tile([P, DC], F32R)
            nc.vector.tensor_copy(bv_r, bv_st)
            bo_st = consts.tile([1, D], F32)
            nc.sync.dma_start(out=bo_st, in_=b_d["bo"].unsqueeze(0))
            bo_r = consts.tile([1, D], F32R)
            nc.vector.tensor_copy(bo_r, bo_st)
            ones_f32 = consts.tile([1, P], F32)
            nc.vector.memset(ones_f32, 1.0)
            ones_row_r = consts.tile([1, P], F32R)
            nc.vector.tensor_copy(ones_row_r, ones_f32)
            cvec_sb = consts.tile([1, D], F32R)
        ones96 = consts.tile([P, SQ * H], F32)
        nc.vector.memset(ones96, 1.0)
        cvec_done = False

        def load_weight(name):
            # DMA fp32 2-chunk pairs through staging, round to f32r on gpsimd
            wr = wpool.tile([P, DC, D], F32R, tag="w", name=f"w_{name}")
            src = w_d[name].rearrange("(c p) m -> p c m", p=P)
            for c in range(0, DC, 2):
                ws = iop.tile([P, 2, D], F32, tag="st2", name=f"ws_{name}_{c}")
                nc.sync.dma_start(out=ws, in_=src[:, c:c + 2, :])
                nc.vector.tensor_copy(wr[:, c:c + 2, :], ws)
            return wr

        for b in range(BL):
            x_b = x_d[b].rearrange("(t p) d -> p t d", p=P)
            y_b = y_d[b].rearrange("(t p) d -> p t d", p=P)

            # ---- phase A: x -> xT (f32r) ----
            xT = big.tile([P, DC, S], F32R, tag="xT", name=f"xT_{b}")
            for sq in range(0, SQ, 2):
                x_in = iop.tile([P, 2, D], F32, tag="st2", name=f"xin_{b}_{sq}")
                nc.sync.dma_start(out=x_in, in_=x_b[:, sq:sq + 2, :])
                for j in range(2):
                    tt = pp.tile([P, 1024], F32, tag="mm",
                                 name=f"tps_{b}_{sq}_{j}")
                    for c in range(DC):
                        nc.tensor.transpose(
                            tt[:, c * P:(c + 1) * P],
                            x_in[:, j, c * P:(c + 1) * P], ident
                        )
                    nc.vector.tensor_copy(
                        xT[:, :, (sq + j) * P:(sq + j + 1) * P],
                        tt[:, :D].rearrange("p (c q) -> p c q", c=DC),
                    )

            if b == 0:
                dump("xT", xT)

            # ---- phase B: projections ----
            wq_r = load_weight("wq")
            QT = big.tile([P, DC, S], F32R, tag="QT", name=f"QT_{b}")
            for m in range(DC):
                qq = pp.tile([P, 1024], F32, tag="mm", name=f"qps_{b}_{m}")
                for c in range(DC):
                    for hf in range(2):
                        nc.tensor.matmul(
                            qq[:, hf * 512:(hf + 1) * 512],
                            wq_r[:, c, m * P:(m + 1) * P],
                            xT[:, c, hf * 512:(hf + 1) * 512],
                            start=(c == 0), stop=(c == DC - 1),
                        )
                if with_bias:
                    nc.vector.tensor_scalar_add(QT[:, m, :], qq,
                                                bq_sb[:, m:m + 1])
                else:
                    nc.vector.tensor_copy(QT[:, m, :], qq)

            wk_r = load_weight("wk")
            KT = big.tile([P, DC, S], F32R, tag="KT", name=f"KT_{b}")
            for m in range(DC):
                kk = pp.tile([P, 1024], F32, tag="mm", name=f"kps_{b}_{m}")
                for c in range(DC):
                    for hf in range(2):
                        nc.tensor.matmul(
                            kk[:, hf * 512:(hf + 1) * 512],
                            wk_r[:, c, m * P:(m + 1) * P],
                            xT[:, c, hf * 512:(hf + 1) * 512],
                            start=(c == 0), stop=(c == DC - 1),
                        )
                if with_bias:
                    nc.vector.tensor_scalar_add(KT[:, m, :], kk,
                                                bk_sb[:, m:m + 1])
                else:
                    nc.vector.tensor_copy(KT[:, m, :], kk)

            wv_r = load_weight("wv")
            # V layout [P, SQ, H, 65]: cols 0..63 = v, col 64 = ones
            V = big.tile([P, SQ, H, 65], F32R, tag="V", name=f"V_{b}")
            nc.vector.tensor_copy(
                V[:, :, :, 64], ones96.rearrange("p (a h) -> p a h", a=SQ)
            )
            for sq in range(SQ):
                vv = pp.tile([P, 1024], F32, tag="mm", name=f"vps_{b}_{sq}")
                for c in range(DC):
                    nc.tensor.matmul(
                        vv[:, 0:512], xT[:, c, sq * P:(sq + 1) * P],
                        wv_r[:, c, 0:512], start=(c == 0), stop=(c == DC - 1),
                    )
                    nc.tensor.matmul(
                        vv[:, 512:D], xT[:, c, sq * P:(sq + 1) * P],
                        wv_r[:, c, 512:D], start=(c == 0), stop=(c == DC - 1),
                    )
                nc.vector.tensor_copy(
                    V[:, sq, :, 0:64],
                    vv[:, :D].rearrange("p (h e) -> p h e", h=H),
                )
            if b == 0:
                dump("QT", QT)
                dump("KT", KT)
                dump("V", V)

            # prefetch output-projection weight during attention
            wo_r = load_weight("wo")
            if with_bias and not cvec_done:
                cvec_done = True
                cv = pp.tile([P, 1024], F32, tag="ov", name="cvps")
                for c in range(DC):
                    nc.tensor.matmul(cv[0:1, 0:512], bv_r[:, c:c + 1],
                                     wo_r[:, c, 0:512], start=(c == 0),
                                     stop=False)
                    nc.tensor.matmul(cv[0:1, 512:D], bv_r[:, c:c + 1],
                                     wo_r[:, c, 512:D], start=(c == 0),
                                     stop=False)
                nc.tensor.matmul(cv[0:1, 0:512], ones_row_r[:, 0:1],
                                 bo_r[:, 0:512], start=False, stop=True)
                nc.tensor.matmul(cv[0:1, 512:D], ones_row_r[:, 0:1],
                                 bo_r[:, 512:D], start=False, stop=True)
                nc.vector.tensor_copy(cvec_sb, cv[0:1, :D])

            # ---- phase C: attention, head pairs interleaved ----
            OTn = big.tile([P, DC, S], F32R, tag="OTn", name=f"OTn_{b}")
            for ch in range(DC):
                oos = [pp.tile([P, 1024], F32, tag="ov",
                               name=f"ops_{b}_{ch}_{par}")
                       for par in range(2)]
                for kt in range(SQ):
                    zzs = []
                    for par in range(2):
                        psl = slice(par * 64, par * 64 + 64)
                        zz = pp.tile([P, 1024], F32, tag="mm",
                                     name=f"zps_{b}_{ch}_{par}_{kt}")
                        ksl = KT[psl, ch, kt * P:(kt + 1) * P]
                        nc.tensor.matmul(zz[:, 0:512], ksl,
                                         QT[psl, ch, 0:512],
                                         start=True, stop=True)
                        nc.tensor.matmul(zz[:, 512:1024], ksl,
                                         QT[psl, ch, 512:1024],
                                         start=True, stop=True)
                        zzs.append(zz)
                    ats = []
                    for par in range(2):
                        at = atp.tile([P, 1024], F32R, tag="at",
                                      name=f"at_{b}_{ch}_{par}_{kt}")
                        nc.scalar.activation(at, zzs[par], EXP, scale=SCALE)
                        ats.append(at)
                        if b == 0 and ch == 0 and par == 0 and kt == 0:
                            dump("at0", at)
                    for par in range(2):
                        vsl = V[:, kt, 2 * ch + par, :]
                        for hf in range(2):
                            nc.tensor.matmul(
                                oos[par][0:65, hf * 512:(hf + 1) * 512],
                                vsl, ats[par][:, hf * 512:(hf + 1) * 512],
                                start=(kt == 0), stop=(kt == SQ - 1),
                            )
                # evict U+denom, then normalize from SBUF
                ues = []
                for par in range(2):
                    ue = smal.tile([65, S], F32, tag="ub",
                                   name=f"ue_{b}_{ch}_{par}",
                                   bufs=(1 if with_bias else 2))
                    nc.vector.tensor_copy(ue, oos[par][0:65, :])
                    ues.append(ue)
                for par in range(2):
                    h = 2 * ch + par
                    psl = slice(par * 64, par * 64 + 64)
                    ue = ues[par]
                    dsl = ue[64:65, :]
                    rbraw = smal.tile([64, S], F32, tag="rbraw",
                                      name=f"rbraw_{b}_{h}", bufs=1)
                    srcap = bass.AP(tensor=dsl.tensor, offset=dsl.offset,
                                    ap=[list(dsl.ap[0]), [0, 64],
                                        list(dsl.ap[1])])
                    nc.gpsimd.dma_start(out=rbraw, in_=srcap)
                    rb = smal.tile([64, S], F32, tag="rb",
                                   name=f"rb_{b}_{h}",
                                   bufs=(1 if with_bias else 2))
                    nc.vector.reciprocal_approx_fast(out=rb, in_=rbraw)
                    if par == 0:
                        nc.vector.tensor_mul(OTn[psl, ch, :], ue[0:64, :], rb)
                    else:
                        stg = smal.tile([64, S], F32R, tag="rbraw",
                                        name=f"stg_{b}_{h}", bufs=1)
                        nc.vector.tensor_mul(stg, ue[0:64, :], rb)
                        nc.gpsimd.dma_start(out=OTn[psl, ch, :], in_=stg)
                    if b == 0 and h == 0:
                        dump("rbi0", rb)
            if b == 0:
                dump("OTn", OTn)

            # ---- phase D/E: output projection ----
            for sq in range(0, SQ, 2):
                yst = iop.tile([P, 2, D], F32, tag="st2", name=f"yst_{b}_{sq}")
                for j in range(2):
                    yy = pp.tile([P, 1024], F32, tag="mm",
                                 name=f"yps_{b}_{sq}_{j}")
                    for c in range(DC):
                        st = OTn[:, c, (sq + j) * P:(sq + j + 1) * P]
                        last = (not with_bias) and c == DC - 1
                        nc.tensor.matmul(yy[:, 0:512], st, wo_r[:, c, 0:512],
                                         start=(c == 0), stop=last)
                        nc.tensor.matmul(yy[:, 512:D], st, wo_r[:, c, 512:D],
                                         start=(c == 0), stop=last)
                    if with_bias:
                        nc.tensor.matmul(yy[:, 0:512], ones_row_r,
                                         cvec_sb[:, 0:512], start=False,
                                         stop=True)
                        nc.tensor.matmul(yy[:, 512:D], ones_row_r,
                                         cvec_sb[:, 512:D], start=False,
                                         stop=True)
                    nc.vector.tensor_copy(yst[:, j, :], yy[:, :D])
                nc.sync.dma_start(out=y_b[:, sq:sq + 2, :], in_=yst)


def _build(with_bias=True):
    nc = bacc.Bacc("TRN2", target_bir_lowering=False, debug=False,
                   num_devices=NCORES)
    x_d = nc.dram_tensor("x", [BL, S, D], F32, kind="ExternalInput").ap()
    w_d = {n: nc.dram_tensor(n, [D, D], F32, kind="ExternalInput").ap()
           for n in ("wq", "wk", "wv", "wo")}
    b_d = {n: nc.dram_tensor(n, [D], F32, kind="ExternalInput").ap()
           for n in ("bq", "bk", "bv", "bo")}
    y_d = nc.dram_tensor("y", [BL, S, D], F32, kind="ExternalOutput").ap()
    dbg = None
    if _DEBUG:
        shapes = {"xT": ([P, DC, S], F32R), "QT": ([P, DC, S], F32R),
                  "KT": ([P, DC, S], F32R), "V": ([P, SQ, H, 65], F32R),
                  "at0": ([P, S], F32R), "rb0": ([65, S], F32),
                  "rbi0": ([64, S], F32),
                  "OTn": ([P, DC, S], F32R)}
        dbg = {n: nc.dram_tensor(f"dbg_{n}", sh, dt,
                                 kind="ExternalOutput").ap()
               for n, (sh, dt) in shapes.items()}
    with tile.TileContext(nc) as tc:
        _emit(tc, x_d, w_d, b_d, y_d, dbg, with_bias=with_bias)
    nc.compile()
    return nc


def _in_maps(x, Wq, bq, Wk, bk, Wv, bv, Wo, bo):
    w = {
        "wq": np.ascontiguousarray(Wq.reshape(D, D), dtype=np.float32),
        "wk": np.ascontiguousarray(Wk.reshape(D, D), dtype=np.float32),
        "wv": np.ascontiguousarray(Wv.reshape(D, D), dtype=np.float32),
        "wo": np.ascontiguousarray(Wo.reshape(D, D), dtype=np.float32),
        "bq": np.ascontiguousarray(bq.reshape(D), dtype=np.float32),
        "bk": np.ascontiguousarray(bk.reshape(D), dtype=np.float32),
        "bv": np.ascontiguousarray(bv.reshape(D), dtype=np.float32),
        "bo": np.ascontiguousarray(bo.reshape(D), dtype=np.float32),
    }
    x = np.asarray(x, dtype=np.float32)
    return [dict(w, x=np.ascontiguousarray(x[i * BL:(i + 1) * BL]))
            for i in range(NCORES)]


def get_nc(with_bias=True):
    if with_bias not in _NC:
        _NC[with_bias] = _build(with_bias=with_bias)
    return _NC[with_bias]


def run(inputs, trace=False):
    with_bias = any(
        np.any(np.asarray(inputs[k])) for k in ("bq", "bk", "bv", "bo"))
    nc = get_nc(with_bias=with_bias)
    maps = _in_maps(**inputs)
    res = run_bass_kernel_spmd(nc, maps, list(range(NCORES)), trace=trace)
    y = np.concatenate([res.results[i]["y"] for i in range(NCORES)], axis=0)
    return y, res


def kernel(x, Wq, bq, Wk, bk, Wv, bv, Wo, bo):
    y, _ = run(dict(x=x, Wq=Wq, bq=bq, Wk=Wk, bk=bk, Wv=Wv, bv=bv,
                    Wo=Wo, bo=bo))
    return y
